# revision 1
# baseline (speedup 1.0000x reference)
"""Trainium2 Bass kernel for a 12-layer GRU LM (embed -> 12x GRU -> vocab decoder).

Strategy (V2): lockstep layer-pipeline across cores, full batch per core.

The recurrent scan is weight-load bound on the PE (~66ns per 128x128 weight
tile regardless of the moving free dim), so data-parallel over batch (each
core runs all 12 layers) wastes 6x. Instead cores 0-5 each own TWO
consecutive GRU layers and the full batch (B=32) flows through a 6-stage
pipeline in chunks of C=8 timesteps. Cores 6-7 run the identical instruction
stream (SPMD) on zero weights; all 8 cores decode their 1/8 vocab shard of
the final layer's output (tensor-parallel decoder), so spectators still do
real decoder work.

Per wave w (lockstep, 22 waves):
  - every core gathers embedding chunk w+1 and writes it (feature-major) to
    staging rows [1024:1152) of the next staging buffer
  - recv: indirect-DMA 128 rows from the current staging buffer at a
    per-core row index (core 0 reads the embedding slot, core c reads core
    c-1's AllGather slot) -- per-core DATA keeps the program uniform
  - per-wave keep-mask multiply zeroes recurrent state right before each
    core's first real chunk (and during garbage waves)
  - slot A: input-side GEMM (N=256) + C-step recurrent scan (layer 2c)
  - slot B: same on slot A's output (layer 2c+1)
  - send slot B output -> AllGather into next staging buffer rows [0:1024)
  - decode staging rows [640:768) (= core 5 slot B = layer 11 output, lagged
    PIPE waves) against this core's vocab shard; garbage waves write to a
    garbage row strip

All GEMMs fp16 (fp32 PSUM); fp8 weights were simulated numerically and fail
(rel err 0.09-0.5 vs the 2e-2 budget: the 128-step recurrence amplifies
weight quantization noise).
"""

import os
import sys

sys.path.insert(0, "/opt/trn_rl_repo")

import contextlib

import numpy as np

import concourse.bass as bass
import concourse.tile as tile
from concourse import bacc, mybir
from concourse.bass_utils import run_bass_kernel_spmd
from concourse.masks import make_identity

F32 = mybir.dt.float32
F16 = mybir.dt.float16
I32 = mybir.dt.int32

# Problem shapes (hardcoded per contract)
VOCAB, H, L, T, B = 30522, 768, 12, 128, 32
N_CORES = 8
JH = H // 128               # 6 feature chunks
G3 = 3 * H // 128           # 18 gate chunks
C = 8                       # timesteps per chunk
TOKC = C * B                # 256 tokens per chunk
NCHUNK = T // C             # 16 chunks
PIPE = 6                    # pipeline depth in cores (2 layers each)
WAVES = NCHUNK + PIPE       # 22 lockstep waves
VPAD = 30720                # vocab padded to 8 * 3840
VS = VPAD // N_CORES        # 3840 vocab shard per core
VC = 480                    # decoder psum chunk (8 per shard)
NTOK = T * B                # 4096 tokens
SROWS = N_CORES * 128 + 128  # staging rows: 8 AG slots + embed slot
SCOLS = JH * TOKC           # 1536
SCAN_UNROLL = 2

_CACHE = {}


def _build():
    nc = bacc.Bacc("TRN2", target_bir_lowering=False, debug=False,
                   num_devices=N_CORES)

    emb = nc.dram_tensor("emb", [VOCAB, H], F16, kind="ExternalInput").ap()
    idsq = nc.dram_tensor("idsq", [128, 2 * NCHUNK], I32, kind="ExternalInput").ap()
    wihT = nc.dram_tensor("wihT", [2, JH, 128, 3 * H], F16, kind="ExternalInput").ap()
    whhT = nc.dram_tensor("whhT", [2, JH, 128, 3 * H], F16, kind="ExternalInput").ap()
    wib = nc.dram_tensor("wib", [2, 1, 3 * H], F16, kind="ExternalInput").ap()
    bhhn = nc.dram_tensor("bhhn", [2, 128, JH], F16, kind="ExternalInput").ap()
    decT = nc.dram_tensor("decT", [JH, 128, VS], F16, kind="ExternalInput").ap()
    decb = nc.dram_tensor("decb", [1, VS], F16, kind="ExternalInput").ap()
    srcidx = nc.dram_tensor("srcidx", [128, 1], I32, kind="ExternalInput").ap()
    keep = nc.dram_tensor("keep", [WAVES, 128, JH * B], F16, kind="ExternalInput").ap()
    out = nc.dram_tensor("out", [NTOK + TOKC, VS], F16, kind="ExternalOutput").ap()

    with tile.TileContext(nc) as tc, contextlib.ExitStack() as ctx:
        const = ctx.enter_context(tc.tile_pool(name="const", bufs=1))
        wpool = ctx.enter_context(tc.tile_pool(name="wpool", bufs=1))
        gpool = ctx.enter_context(tc.tile_pool(name="gpool", bufs=1))
        spool = ctx.enter_context(tc.tile_pool(name="spool", bufs=1))
        xpool = ctx.enter_context(tc.tile_pool(name="xpool", bufs=1))
        dpool = ctx.enter_context(tc.tile_pool(name="dpool", bufs=2))
        epool = ctx.enter_context(tc.tile_pool(name="epool", bufs=2))
        stpool = ctx.enter_context(tc.tile_pool(name="stpool", bufs=3))
        ps = ctx.enter_context(tc.tile_pool(name="ps", bufs=2, space="PSUM"))
        psg = ctx.enter_context(tc.tile_pool(name="psg", bufs=2, space="PSUM"))
        psd = ctx.enter_context(tc.tile_pool(name="psd", bufs=1, space="PSUM"))
        pse = ctx.enter_context(tc.tile_pool(name="pse", bufs=1, space="PSUM"))
        dram = ctx.enter_context(tc.tile_pool(name="dram", bufs=1, space="DRAM"))

        # staging buffers (double-buffered across waves) + AG input bounce
        S0 = dram.tile([SROWS, SCOLS], F16)
        S1 = dram.tile([SROWS, SCOLS], F16)
        Sbuf = [S0, S1]
        agin0 = dram.tile([128, SCOLS], F16)
        agin1 = dram.tile([128, SCOLS], F16)
        aginb = [agin0, agin1]

        # ---- constants / weights ----
        ident = const.tile([128, 128], F16)
        make_identity(nc, ident[:])
        ones = const.tile([1, TOKC], F16)
        nc.vector.memset(ones[:], 1.0)
        ids_sb = const.tile([128, 2 * NCHUNK], I32)
        nc.sync.dma_start(ids_sb[:], idsq[:])
        srcidx_sb = const.tile([128, 1], I32)
        nc.sync.dma_start(srcidx_sb[:], srcidx[:])

        wih_sb = const.tile([128, 2, JH, 3 * H], F16)
        whh_sb = const.tile([128, 2, JH, 3 * H], F16)
        wib_sb = const.tile([1, 2, 3 * H], F16)
        for s in range(2):
            for j in range(JH):
                nc.sync.dma_start(wih_sb[:, s, j, :], wihT[s, j])
                nc.sync.dma_start(whh_sb[:, s, j, :], whhT[s, j])
            nc.sync.dma_start(wib_sb[:, s, :], wib[s])
        bhhn_sb = wpool.tile([128, 2, JH, 1], F16)
        bhhn_bc = wpool.tile([128, 2, JH, B], F16)
        for s in range(2):
            nc.sync.dma_start(bhhn_sb[:, s, :, 0], bhhn[s])
            for b in range(B):
                nc.vector.tensor_copy(bhhn_bc[:, s, :, b:b + 1], bhhn_sb[:, s])
        decb_sb = const.tile([1, VS], F16)
        nc.sync.dma_start(decb_sb[:], decb[:])

        # zero both staging buffers so garbage waves stay finite
        zt = wpool.tile([128, SCOLS], F16)
        nc.vector.memset(zt[:], 0.0)
        for Sb in Sbuf:
            for r in range(SROWS // 128):
                nc.sync.dma_start(Sb[r * 128:(r + 1) * 128, :], zt[:])

        # ---- working tiles ----
        xin = xpool.tile([128, JH, C, B], F16)        # received chunk
        xoutA = xpool.tile([128, JH, C, B], F16)
        xoutB = xpool.tile([128, JH, C, B], F16)
        xdec = xpool.tile([128, JH, TOKC], F16)
        embfm = xpool.tile([128, JH, TOKC], F16)
        giA = gpool.tile([128, G3, C, B], F16)
        giB = gpool.tile([128, G3, C, B], F16)
        keep_sb = spool.tile([128, JH, B], F16)

        # per-slot scan state + DVE chain (f16, slot on a free axis)
        hbf = spool.tile([128, 2, JH, B], F16)
        rz = spool.tile([128, 2, 2 * JH, B], F16)
        rzs = spool.tile([128, 2, 2 * JH, B], F16)
        ghn = spool.tile([128, 2, JH, B], F16)
        npre = spool.tile([128, 2, JH, B], F16)
        nts = spool.tile([128, 2, JH, B], F16)
        nt = spool.tile([128, 2, JH, B], F16)
        dd = spool.tile([128, 2, JH, B], F16)
        zd = spool.tile([128, 2, JH, B], F16)
        nc.vector.memset(hbf[:], 0.0)

        def gi_slot(s, gi, x):
            # gi[g, t, b] = sum_j wih[s]_j_g^T x_j + bias ; N=TOKC GEMMs
            xf = x[:].rearrange("p j t b -> p j (t b)")
            gf = gi[:].rearrange("p g t b -> p g (t b)")
            for g in range(G3):
                pg = psg.tile([128, TOKC], F32, tag="gips")
                for j in range(JH):
                    nc.tensor.matmul(pg[:], wih_sb[:, s, j, g * 128:(g + 1) * 128],
                                     xf[:, j, :], start=(j == 0), stop=False)
                nc.tensor.matmul(pg[:], wib_sb[0:1, s, g * 128:(g + 1) * 128],
                                 ones[0:1, :], start=False, stop=True)
                nc.vector.tensor_copy(gf[:, g, :], pg[:])

        def scan_slot(s, gi, xout):
            def step(t):
                pgh_rz = ps.tile([128, 2 * JH, B], F32, tag="ghps_rz")
                pgh_n = ps.tile([128, JH, B], F32, tag="ghps_n")
                for g in range(2 * JH):
                    for j in range(JH):
                        nc.tensor.matmul(pgh_rz[:, g, :],
                                         whh_sb[:, s, j, g * 128:(g + 1) * 128],
                                         hbf[:, s, j, :],
                                         start=(j == 0), stop=(j == JH - 1))
                for g in range(2 * JH, G3):
                    for j in range(JH):
                        nc.tensor.matmul(pgh_n[:, g - 2 * JH, :],
                                         whh_sb[:, s, j, g * 128:(g + 1) * 128],
                                         hbf[:, s, j, :],
                                         start=(j == 0), stop=(j == JH - 1))
                nc.vector.tensor_add(rz[:, s], pgh_rz[:], gi[:, 0:2 * JH, t, :])
                nc.scalar.activation(rzs[:, s], rz[:, s],
                                     mybir.ActivationFunctionType.Sigmoid)
                nc.vector.tensor_add(ghn[:, s], pgh_n[:], bhhn_bc[:, s])
                nc.vector.tensor_mul(npre[:, s], rzs[:, s, 0:JH, :], ghn[:, s])
                nc.vector.tensor_add(npre[:, s], npre[:, s], gi[:, 2 * JH:G3, t, :])
                nc.scalar.activation(nt[:, s], npre[:, s],
                                     mybir.ActivationFunctionType.Tanh)
                nc.vector.tensor_sub(dd[:, s], hbf[:, s], nt[:, s])
                nc.vector.tensor_mul(zd[:, s], rzs[:, s, JH:2 * JH, :], dd[:, s])
                nc.vector.tensor_add(hbf[:, s], zd[:, s], nt[:, s])
                nc.vector.tensor_copy(xout[:, :, t, :], hbf[:, s])

            with tc.For_i(0, C, SCAN_UNROLL,
                          hint_engines=(mybir.EngineType.PE,
                                        mybir.EngineType.DVE)) as t0:
                for dt in range(SCAN_UNROLL):
                    step(t0 + dt)

        def embed_gather(chunk, Sdst):
            for grp in range(2):
                g = epool.tile([128, H], F16, tag="egather")
                col = chunk * 2 + grp
                nc.gpsimd.indirect_dma_start(
                    out=g[:], out_offset=None, in_=emb[:],
                    in_offset=bass.IndirectOffsetOnAxis(
                        ap=ids_sb[:, col:col + 1], axis=0),
                )
                for j in range(JH):
                    tp = pse.tile([128, 128], F16, tag="trps")
                    nc.tensor.transpose(out=tp[:], in_=g[:, j * 128:(j + 1) * 128],
                                        identity=ident[:])
                    nc.vector.tensor_copy(
                        embfm[:, j, grp * 128:(grp + 1) * 128], tp[:])
            nc.sync.dma_start(Sdst[N_CORES * 128:N_CORES * 128 + 128, :],
                              embfm[:].rearrange("p j t -> p (j t)"))

        # prologue: chunk 0 embedding into S0 (read by core 0 at wave 0)
        embed_gather(0, S0)

        for w in range(WAVES):
            Scur = Sbuf[w % 2]
            Snxt = Sbuf[(w + 1) % 2]

            # ---- embedding gather for chunk w+1 -> Snxt embed slot ----
            embed_gather(min(w + 1, NCHUNK - 1), Snxt)

            # ---- recv x chunk (per-core source row indices) ----
            nc.gpsimd.indirect_dma_start(
                out=xin[:].rearrange("p j t b -> p (j t b)"), out_offset=None,
                in_=Scur[:],
                in_offset=bass.IndirectOffsetOnAxis(ap=srcidx_sb[:, 0:1], axis=0),
            )

            # ---- keep-mask: zero recurrent state outside the real window ----
            nc.sync.dma_start(keep_sb[:].rearrange("p j b -> p (j b)"), keep[w])
            for s in range(2):
                nc.vector.tensor_mul(hbf[:, s], hbf[:, s], keep_sb[:])

            # ---- two pipeline slots ----
            gi_slot(0, giA, xin)
            scan_slot(0, giA, xoutA)
            gi_slot(1, giB, xoutA)
            scan_slot(1, giB, xoutB)

            # ---- send + AllGather into next staging ----
            agin = aginb[w % 2]
            nc.sync.dma_start(agin[:], xoutB[:].rearrange("p j t b -> p (j t b)"))
            nc.gpsimd.collective_compute(
                "AllGather", mybir.AluOpType.bypass,
                replica_groups=[list(range(N_CORES))],
                ins=[agin.opt()], outs=[Snxt[0:N_CORES * 128, :]],
            )

            # ---- decode layer-11 chunk (lagged PIPE waves) ----
            nc.sync.dma_start(xdec[:].rearrange("p j t -> p (j t)"),
                              Scur[5 * 128:6 * 128, :])
            rc = w - PIPE
            orow = rc * TOKC if 0 <= rc < NCHUNK else NTOK
            for vc in range(VS // VC):
                dwt = dpool.tile([128, JH, VC], F16, tag="decw")
                for j in range(JH):
                    nc.sync.dma_start(dwt[:, j, :],
                                      decT[j, :, vc * VC:(vc + 1) * VC])
                for tg in range(2):
                    pd = psd.tile([128, VC], F32, tag="decps")
                    for j in range(JH):
                        nc.tensor.matmul(pd[:], xdec[:, j, tg * 128:(tg + 1) * 128],
                                         dwt[:, j, :], start=(j == 0), stop=False)
                    nc.tensor.matmul(pd[:], ones[0:1, 0:128],
                                     decb_sb[0:1, vc * VC:(vc + 1) * VC],
                                     start=False, stop=True)
                    stage = stpool.tile([128, VC], F16, tag="stage")
                    nc.vector.tensor_copy(stage[:], pd[:])
                    nc.sync.dma_start(
                        out[orow + tg * 128:orow + (tg + 1) * 128,
                            vc * VC:(vc + 1) * VC], stage[:])

    nc.compile()
    return nc


def _prep_inputs(input_ids, embedding, w_ih, w_hh, b_ih, b_hh, dec_w, dec_b):
    f16 = np.float16
    emb_np = np.ascontiguousarray(embedding.astype(f16))

    # ids: [T, B] -> [128, 2*NCHUNK]; col = chunk*2+grp, row p = token grp*128+p
    ids32 = np.asarray(input_ids).astype(np.int32).reshape(NCHUNK, C * B)
    idsq_np = np.ascontiguousarray(
        ids32.reshape(NCHUNK, 2, 128).transpose(2, 0, 1).reshape(128, 2 * NCHUNK))

    wihT_all = w_ih.transpose(0, 2, 1).reshape(L, JH, 128, 3 * H).astype(f16)
    whhT_all = w_hh.transpose(0, 2, 1).reshape(L, JH, 128, 3 * H).astype(f16)
    wib_all = b_ih.copy()
    wib_all[:, :2 * H] += b_hh[:, :2 * H]
    wib_all = wib_all.reshape(L, 1, 3 * H).astype(f16)
    bhhn_all = b_hh[:, 2 * H:].reshape(L, JH, 128).transpose(0, 2, 1).astype(f16)

    decT_full = np.zeros((JH, 128, VPAD), dtype=f16)
    decT_full[:, :, :VOCAB] = dec_w.T.reshape(JH, 128, VOCAB).astype(f16)
    decb_full = np.zeros((1, VPAD), dtype=f16)
    decb_full[0, :VOCAB] = dec_b.astype(f16)

    in_maps = []
    for c in range(N_CORES):
        if c < PIPE:
            l0, l1 = 2 * c, 2 * c + 1
            wih_np = np.ascontiguousarray(wihT_all[[l0, l1]])
            whh_np = np.ascontiguousarray(whhT_all[[l0, l1]])
            wib_np = np.ascontiguousarray(wib_all[[l0, l1]])
            bhhn_np = np.ascontiguousarray(bhhn_all[[l0, l1]])
        else:
            wih_np = np.zeros((2, JH, 128, 3 * H), dtype=f16)
            whh_np = np.zeros((2, JH, 128, 3 * H), dtype=f16)
            wib_np = np.zeros((2, 1, 3 * H), dtype=f16)
            bhhn_np = np.zeros((2, 128, JH), dtype=f16)

        base = N_CORES * 128 if c == 0 else (c - 1) * 128
        srcidx_np = (base + np.arange(128, dtype=np.int32)).reshape(128, 1)

        keep_np = np.zeros((WAVES, 128, JH * B), dtype=f16)
        if c < PIPE:
            keep_np[c + 1:c + NCHUNK] = 1.0

        m = {
            "emb": emb_np, "idsq": idsq_np,
            "wihT": wih_np, "whhT": whh_np, "wib": wib_np, "bhhn": bhhn_np,
            "decT": np.ascontiguousarray(decT_full[:, :, c * VS:(c + 1) * VS]),
            "decb": np.ascontiguousarray(decb_full[:, c * VS:(c + 1) * VS]),
            "srcidx": srcidx_np, "keep": keep_np,
        }
        in_maps.append(m)
    return in_maps


def kernel(input_ids, embedding, w_ih, w_hh, b_ih, b_hh, dec_w, dec_b):
    if "nc" not in _CACHE:
        _CACHE["nc"] = _build()
    nc = _CACHE["nc"]
    in_maps = _prep_inputs(input_ids, embedding, w_ih, w_hh, b_ih, b_hh,
                           dec_w, dec_b)
    res = run_bass_kernel_spmd(nc, in_maps, core_ids=list(range(N_CORES)))
    full = np.empty((T, B, VPAD), dtype=np.float32)
    for c in range(N_CORES):
        o = res.results[c]["out"][:NTOK].astype(np.float32)
        full[:, :, c * VS:(c + 1) * VS] = o.reshape(T, B, VS)
    return full[:, :, :VOCAB]


if __name__ == "__main__":
    _build()
    print("build OK")



# revision 4
# speedup vs baseline: 1.3410x; 1.3410x over previous
"""Trainium2 Bass kernel for a 12-layer GRU LM (embed -> 12x GRU -> vocab decoder).

Strategy (V3): lockstep layer-pipeline across cores, full batch per core,
with a PE-dense wave schedule.

Cores 0-5 each own TWO consecutive GRU layers; the full batch (B=32) flows
through a 6-stage pipeline in chunks of C=8 timesteps (22 lockstep waves).
Cores 6-7 run the identical instruction stream (SPMD) on zero weights; all 8
cores decode their 1/8 vocab shard of the final layer's output.

V3 changes vs V2 (5.06 ms baseline):
  - Scan fully unrolled (no For_i); after each scan step's matmuls one
    decoder PSUM-group (~1.8us of independent matmuls) is issued so the PE
    stays busy during the step's serial DVE/ACT chain. This both recovers
    ~2 ms of PE idle and keeps the HAM clock un-throttled (the 2-6us chain
    gaps were re-throttling the PE to 1.2 GHz for 73% of the kernel).
  - rz-gate matmuls issue before n-gate matmuls within a step so the chain
    (rz add + sigmoid) overlaps the n-matmul stream.
  - h written directly into xout[:, :, t, :] (previous step read from t-1),
    killing the per-step DVE copy.
  - Staging buffers allocated addr_space="Shared" for the fast HBM-HBM
    AllGather path.

All GEMMs fp16 (fp32 PSUM); fp8 weights fail numerically for the recurrence
(rel err 0.09-0.5 vs the 2e-2 budget).
"""

import os
import sys

sys.path.insert(0, "/opt/trn_rl_repo")

import contextlib

import numpy as np

import concourse.bass as bass
import concourse.tile as tile
from concourse import bacc, mybir
from concourse.bass_utils import run_bass_kernel_spmd
from concourse.masks import make_identity

F32 = mybir.dt.float32
F16 = mybir.dt.float16
I32 = mybir.dt.int32

# Problem shapes (hardcoded per contract)
VOCAB, H, L, T, B = 30522, 768, 12, 128, 32
N_CORES = 8
JH = H // 128               # 6 feature chunks
G3 = 3 * H // 128           # 18 gate chunks
GRZ = 2 * H // 128          # 12 rz gate chunks
C = 8                       # timesteps per chunk
TOKC = C * B                # 256 tokens per chunk
NCHUNK = T // C             # 16 chunks
PIPE = 6                    # pipeline depth in cores (2 layers each)
WAVES = NCHUNK + PIPE       # 22 lockstep waves
VPAD = 30720                # vocab padded to 8 * 3840
VS = VPAD // N_CORES        # 3840 vocab shard per core
VC = 480                    # decoder psum chunk (8 per shard)
NTOK = T * B                # 4096 tokens
SROWS = N_CORES * 128 + 128  # staging rows: 8 AG slots + embed slot
SCOLS = JH * TOKC           # 1536

_CACHE = {}


def _build():
    nc = bacc.Bacc("TRN2", target_bir_lowering=False, debug=False,
                   num_devices=N_CORES)

    emb = nc.dram_tensor("emb", [VOCAB, H], F16, kind="ExternalInput").ap()
    idsq = nc.dram_tensor("idsq", [128, 2 * NCHUNK], I32, kind="ExternalInput").ap()
    wihT = nc.dram_tensor("wihT", [2, JH, 128, 3 * H], F16, kind="ExternalInput").ap()
    whhT = nc.dram_tensor("whhT", [2, JH, 128, 3 * H], F16, kind="ExternalInput").ap()
    wib = nc.dram_tensor("wib", [2, 1, 3 * H], F16, kind="ExternalInput").ap()
    bhhn = nc.dram_tensor("bhhn", [2, 128, JH], F16, kind="ExternalInput").ap()
    decT = nc.dram_tensor("decT", [JH, 128, VS], F16, kind="ExternalInput").ap()
    decb = nc.dram_tensor("decb", [1, VS], F16, kind="ExternalInput").ap()
    srcidx = nc.dram_tensor("srcidx", [128, 1], I32, kind="ExternalInput").ap()
    keep = nc.dram_tensor("keep", [WAVES, 128, JH * B], F16, kind="ExternalInput").ap()
    out = nc.dram_tensor("out", [NTOK + TOKC, VS], F16, kind="ExternalOutput").ap()

    with tile.TileContext(nc) as tc, contextlib.ExitStack() as ctx:
        const = ctx.enter_context(tc.tile_pool(name="const", bufs=1))
        wpool = ctx.enter_context(tc.tile_pool(name="wpool", bufs=1))
        gpool = ctx.enter_context(tc.tile_pool(name="gpool", bufs=1))
        spool = ctx.enter_context(tc.tile_pool(name="spool", bufs=1))
        xpool = ctx.enter_context(tc.tile_pool(name="xpool", bufs=1))
        dpool = ctx.enter_context(tc.tile_pool(name="dpool", bufs=3))
        epool = ctx.enter_context(tc.tile_pool(name="epool", bufs=2))
        stpool = ctx.enter_context(tc.tile_pool(name="stpool", bufs=3))
        ps = ctx.enter_context(tc.tile_pool(name="ps", bufs=1, space="PSUM"))
        psg = ctx.enter_context(tc.tile_pool(name="psg", bufs=2, space="PSUM"))
        psd = ctx.enter_context(tc.tile_pool(name="psd", bufs=2, space="PSUM"))
        pse = ctx.enter_context(tc.tile_pool(name="pse", bufs=2, space="PSUM"))
        dram = ctx.enter_context(tc.tile_pool(name="dram", bufs=1, space="DRAM"))

        # staging buffers (double-buffered across waves) + AG input bounce
        S0 = dram.tile([SROWS, SCOLS], F16)
        S1 = dram.tile([SROWS, SCOLS], F16)
        Sbuf = [S0, S1]
        agin0 = dram.tile([128, SCOLS], F16)
        agin1 = dram.tile([128, SCOLS], F16)
        aginb = [agin0, agin1]

        # ---- constants / weights ----
        ident = const.tile([128, 128], F16)
        make_identity(nc, ident[:])
        ones = const.tile([1, TOKC], F16)
        nc.vector.memset(ones[:], 1.0)
        ids_sb = const.tile([128, 2 * NCHUNK], I32)
        nc.sync.dma_start(ids_sb[:], idsq[:])
        srcidx_sb = const.tile([128, 1], I32)
        nc.sync.dma_start(srcidx_sb[:], srcidx[:])

        wih_sb = const.tile([128, 2, JH, 3 * H], F16)
        whh_sb = const.tile([128, 2, JH, 3 * H], F16)
        wib_sb = const.tile([1, 2, 3 * H], F16)
        for s in range(2):
            for j in range(JH):
                nc.sync.dma_start(wih_sb[:, s, j, :], wihT[s, j])
                nc.sync.dma_start(whh_sb[:, s, j, :], whhT[s, j])
            nc.sync.dma_start(wib_sb[:, s, :], wib[s])
        bhhn_sb = wpool.tile([128, 2, JH, 1], F16)
        bhhn_bc = wpool.tile([128, 2, JH, B], F16)
        for s in range(2):
            nc.sync.dma_start(bhhn_sb[:, s, :, 0], bhhn[s])
            for b in range(B):
                nc.vector.tensor_copy(bhhn_bc[:, s, :, b:b + 1], bhhn_sb[:, s])
        decb_sb = const.tile([1, VS], F16)
        nc.sync.dma_start(decb_sb[:], decb[:])

        # zero both staging buffers so garbage waves stay finite
        zt = wpool.tile([128, SCOLS], F16)
        nc.vector.memset(zt[:], 0.0)
        for Sb in Sbuf:
            for r in range(SROWS // 128):
                nc.sync.dma_start(Sb[r * 128:(r + 1) * 128, :], zt[:])

        # ---- working tiles ----
        xin = xpool.tile([128, JH, C, B], F16)        # received chunk
        xoutA = xpool.tile([128, JH, C, B], F16)
        xoutB = xpool.tile([128, JH, C, B], F16)
        xdec = xpool.tile([128, JH, TOKC], F16)
        embfm = xpool.tile([128, JH, TOKC], F16)
        giA = gpool.tile([128, G3, C, B], F16)
        giB = gpool.tile([128, G3, C, B], F16)
        keep_sb = spool.tile([128, JH, B], F16)

        # per-slot chain tiles (slot on a free axis; A/B slices never conflict)
        rz = spool.tile([128, 2, GRZ, B], F16)
        rzs = spool.tile([128, 2, GRZ, B], F16)
        ghn = spool.tile([128, 2, JH, B], F16)
        npre = spool.tile([128, 2, JH, B], F16)
        nt = spool.tile([128, 2, JH, B], F16)
        dd = spool.tile([128, 2, JH, B], F16)
        zd = spool.tile([128, 2, JH, B], F16)
        nc.vector.memset(xoutA[:], 0.0)
        nc.vector.memset(xoutB[:], 0.0)

        xouts = [xoutA, xoutB]
        gis = [giA, giB]

        def gi_slot(s, gi, x):
            # gi[g, t, b] = sum_j wih[s]_j_g^T x_j + bias ; N=TOKC GEMMs
            xf = x[:].rearrange("p j t b -> p j (t b)")
            gf = gi[:].rearrange("p g t b -> p g (t b)")
            for g in range(G3):
                pg = psg.tile([128, TOKC], F32, tag="gips")
                for j in range(JH):
                    nc.tensor.matmul(pg[:], wih_sb[:, s, j, g * 128:(g + 1) * 128],
                                     xf[:, j, :], start=(j == 0), stop=False)
                nc.tensor.matmul(pg[:], wib_sb[0:1, s, g * 128:(g + 1) * 128],
                                 ones[0:1, :], start=False, stop=True)
                nc.vector.tensor_copy(gf[:, g, :], pg[:])

        def scan_step(s, gi, xout, t):
            # h_prev: previous timestep's output (t-1), or last step of the
            # previous wave's chunk (masked at wave start) for t == 0.
            hp = xout[:, :, (t - 1) % C, :]
            pgh_rz = ps.tile([128, GRZ, B], F32, tag="ghps_rz")
            pgh_n = ps.tile([128, JH, B], F32, tag="ghps_n")
            for g in range(GRZ):
                for j in range(JH):
                    nc.tensor.matmul(pgh_rz[:, g, :],
                                     whh_sb[:, s, j, g * 128:(g + 1) * 128],
                                     hp[:, j, :],
                                     start=(j == 0), stop=(j == JH - 1))
            for g in range(GRZ, G3):
                for j in range(JH):
                    nc.tensor.matmul(pgh_n[:, g - GRZ, :],
                                     whh_sb[:, s, j, g * 128:(g + 1) * 128],
                                     hp[:, j, :],
                                     start=(j == 0), stop=(j == JH - 1))
            # serial chain: overlaps the n-matmuls / following dec group
            nc.vector.tensor_add(rz[:, s], pgh_rz[:], gi[:, 0:GRZ, t, :])
            nc.scalar.activation(rzs[:, s], rz[:, s],
                                 mybir.ActivationFunctionType.Sigmoid)
            nc.vector.tensor_add(ghn[:, s], pgh_n[:], bhhn_bc[:, s])
            nc.vector.tensor_mul(npre[:, s], rzs[:, s, 0:JH, :], ghn[:, s])
            nc.vector.tensor_add(npre[:, s], npre[:, s], gi[:, GRZ:G3, t, :])
            nc.scalar.activation(nt[:, s], npre[:, s],
                                 mybir.ActivationFunctionType.Tanh)
            nc.vector.tensor_sub(dd[:, s], hp, nt[:, s])
            nc.vector.tensor_mul(zd[:, s], rzs[:, s, JH:GRZ, :], dd[:, s])
            nc.vector.tensor_add(xout[:, :, t, :], zd[:, s], nt[:, s])

        def dec_group(vc, tg, orow):
            # one decoder psum group: ~1.8us of PE work, fills a chain gap
            dwt = dec_w_tiles[vc]
            pd = psd.tile([128, VC], F32, tag="decps")
            for j in range(JH):
                nc.tensor.matmul(pd[:], xdec[:, j, tg * 128:(tg + 1) * 128],
                                 dwt[:, j, :], start=(j == 0), stop=False)
            nc.tensor.matmul(pd[:], ones[0:1, 0:128],
                             decb_sb[0:1, vc * VC:(vc + 1) * VC],
                             start=False, stop=True)
            stage = stpool.tile([128, VC], F16, tag="stage")
            nc.vector.tensor_copy(stage[:], pd[:])
            nc.sync.dma_start(
                out[orow + tg * 128:orow + (tg + 1) * 128,
                    vc * VC:(vc + 1) * VC], stage[:])

        def dec_load(vc):
            dwt = dpool.tile([128, JH, VC], F16, tag="decw")
            for j in range(JH):
                nc.sync.dma_start(dwt[:, j, :],
                                  decT[j, :, vc * VC:(vc + 1) * VC])
            return dwt

        def embed_gather(chunk, Sdst):
            # indirect-gather 2x128 token embeddings; transposes issued
            # separately (embed_transposes) so they can fill PE gaps
            tiles = []
            for grp in range(2):
                g = epool.tile([128, H], F16, tag="egather")
                col = chunk * 2 + grp
                nc.gpsimd.indirect_dma_start(
                    out=g[:], out_offset=None, in_=emb[:],
                    in_offset=bass.IndirectOffsetOnAxis(
                        ap=ids_sb[:, col:col + 1], axis=0),
                )
                tiles.append(g)
            return tiles

        def embed_transposes(tiles, Sdst):
            for grp in range(2):
                g = tiles[grp]
                for j in range(JH):
                    tp = pse.tile([128, 128], F16, tag="trps")
                    nc.tensor.transpose(out=tp[:], in_=g[:, j * 128:(j + 1) * 128],
                                        identity=ident[:])
                    nc.vector.tensor_copy(
                        embfm[:, j, grp * 128:(grp + 1) * 128], tp[:])
            nc.sync.dma_start(Sdst[N_CORES * 128:N_CORES * 128 + 128, :],
                              embfm[:].rearrange("p j t -> p (j t)"))

        # prologue: chunk 0 embedding into S0 (read by core 0 at wave 0)
        etiles = embed_gather(0, S0)
        embed_transposes(etiles, S0)

        for w in range(WAVES):
            Scur = Sbuf[w % 2]
            Snxt = Sbuf[(w + 1) % 2]

            # ---- embedding gather for chunk w+1 (DMA only; transposes later)
            etiles = embed_gather(min(w + 1, NCHUNK - 1), Snxt)

            # ---- recv x chunk (per-core source row indices) ----
            nc.gpsimd.indirect_dma_start(
                out=xin[:].rearrange("p j t b -> p (j t b)"), out_offset=None,
                in_=Scur[:],
                in_offset=bass.IndirectOffsetOnAxis(ap=srcidx_sb[:, 0:1], axis=0),
            )

            # ---- dec input for this wave (layer-11 output, lagged PIPE waves)
            nc.sync.dma_start(xdec[:].rearrange("p j t -> p (j t)"),
                              Scur[5 * 128:6 * 128, :])
            rc = w - PIPE
            orow = rc * TOKC if 0 <= rc < NCHUNK else NTOK

            # ---- keep-mask: zero recurrent state outside the real window ----
            nc.sync.dma_start(keep_sb[:].rearrange("p j b -> p (j b)"), keep[w])
            for s in range(2):
                nc.vector.tensor_mul(xouts[s][:, :, C - 1, :],
                                     xouts[s][:, :, C - 1, :], keep_sb[:])

            # prefetch first dec weight chunks
            dec_w_tiles = {}
            dec_w_tiles[0] = dec_load(0)
            dec_w_tiles[1] = dec_load(1)

            # ---- slot A: input-side GEMM then C-step scan w/ dec fillers ----
            gi_slot(0, giA, xin)
            dec_seq = [(vc, tg) for vc in range(VS // VC) for tg in range(2)]
            di = 0
            for t in range(C):
                scan_step(0, giA, xoutA, t)
                vc, tg = dec_seq[di]
                dec_group(vc, tg, orow)
                di += 1
                if tg == 1 and vc + 2 <= VS // VC - 1:
                    dec_w_tiles[vc + 2] = dec_load(vc + 2)

            # ---- slot B on slot A's output ----
            gi_slot(1, giB, xoutA)
            for t in range(C):
                scan_step(1, giB, xoutB, t)
                vc, tg = dec_seq[di]
                dec_group(vc, tg, orow)
                di += 1
                if tg == 1 and vc + 2 <= VS // VC - 1:
                    dec_w_tiles[vc + 2] = dec_load(vc + 2)

            # ---- send + AllGather into next staging ----
            agin = aginb[w % 2]
            nc.sync.dma_start(agin[:], xoutB[:].rearrange("p j t b -> p (j t b)"))
            nc.gpsimd.collective_compute(
                "AllGather", mybir.AluOpType.bypass,
                replica_groups=[list(range(N_CORES))],
                ins=[agin.opt()], outs=[Snxt[0:N_CORES * 128, :]],
            )

            # ---- embed transposes: PE work covering the AllGather window ----
            embed_transposes(etiles, Snxt)

    nc.compile()
    return nc


def _prep_inputs(input_ids, embedding, w_ih, w_hh, b_ih, b_hh, dec_w, dec_b):
    f16 = np.float16
    emb_np = np.ascontiguousarray(embedding.astype(f16))

    # ids: [T, B] -> [128, 2*NCHUNK]; col = chunk*2+grp, row p = token grp*128+p
    ids32 = np.asarray(input_ids).astype(np.int32).reshape(NCHUNK, C * B)
    idsq_np = np.ascontiguousarray(
        ids32.reshape(NCHUNK, 2, 128).transpose(2, 0, 1).reshape(128, 2 * NCHUNK))

    wihT_all = w_ih.transpose(0, 2, 1).reshape(L, JH, 128, 3 * H).astype(f16)
    whhT_all = w_hh.transpose(0, 2, 1).reshape(L, JH, 128, 3 * H).astype(f16)
    wib_all = b_ih.copy()
    wib_all[:, :2 * H] += b_hh[:, :2 * H]
    wib_all = wib_all.reshape(L, 1, 3 * H).astype(f16)
    bhhn_all = b_hh[:, 2 * H:].reshape(L, JH, 128).transpose(0, 2, 1).astype(f16)

    decT_full = np.zeros((JH, 128, VPAD), dtype=f16)
    decT_full[:, :, :VOCAB] = dec_w.T.reshape(JH, 128, VOCAB).astype(f16)
    decb_full = np.zeros((1, VPAD), dtype=f16)
    decb_full[0, :VOCAB] = dec_b.astype(f16)

    in_maps = []
    for c in range(N_CORES):
        if c < PIPE:
            l0, l1 = 2 * c, 2 * c + 1
            wih_np = np.ascontiguousarray(wihT_all[[l0, l1]])
            whh_np = np.ascontiguousarray(whhT_all[[l0, l1]])
            wib_np = np.ascontiguousarray(wib_all[[l0, l1]])
            bhhn_np = np.ascontiguousarray(bhhn_all[[l0, l1]])
        else:
            wih_np = np.zeros((2, JH, 128, 3 * H), dtype=f16)
            whh_np = np.zeros((2, JH, 128, 3 * H), dtype=f16)
            wib_np = np.zeros((2, 1, 3 * H), dtype=f16)
            bhhn_np = np.zeros((2, 128, JH), dtype=f16)

        base = N_CORES * 128 if c == 0 else (c - 1) * 128
        srcidx_np = (base + np.arange(128, dtype=np.int32)).reshape(128, 1)

        keep_np = np.zeros((WAVES, 128, JH * B), dtype=f16)
        if c < PIPE:
            keep_np[c + 1:c + NCHUNK] = 1.0

        m = {
            "emb": emb_np, "idsq": idsq_np,
            "wihT": wih_np, "whhT": whh_np, "wib": wib_np, "bhhn": bhhn_np,
            "decT": np.ascontiguousarray(decT_full[:, :, c * VS:(c + 1) * VS]),
            "decb": np.ascontiguousarray(decb_full[:, c * VS:(c + 1) * VS]),
            "srcidx": srcidx_np, "keep": keep_np,
        }
        in_maps.append(m)
    return in_maps


def kernel(input_ids, embedding, w_ih, w_hh, b_ih, b_hh, dec_w, dec_b):
    if "nc" not in _CACHE:
        _CACHE["nc"] = _build()
    nc = _CACHE["nc"]
    in_maps = _prep_inputs(input_ids, embedding, w_ih, w_hh, b_ih, b_hh,
                           dec_w, dec_b)
    res = run_bass_kernel_spmd(nc, in_maps, core_ids=list(range(N_CORES)))
    full = np.empty((T, B, VPAD), dtype=np.float32)
    for c in range(N_CORES):
        o = res.results[c]["out"][:NTOK].astype(np.float32)
        full[:, :, c * VS:(c + 1) * VS] = o.reshape(T, B, VS)
    return full[:, :, :VOCAB]


if __name__ == "__main__":
    _build()
    print("build OK")
